# revision 8
# baseline (speedup 1.0000x reference)
"""Trainium2 kernel for nn_CascadedABCDCircuit: cascaded 2-port ABCD ladder.

Math: each stage multiplies the ABCD state by (I + s_i*G_i) with G_i nilpotent,
so every output component is a Laurent polynomial in omega (degree -6..+6, 13
coefficients) with batch-dependent coefficients. Host computes the coefficients
exactly in fp64 (tiny (1024,13) complex recurrence); the device evaluates
out[c,b,f] = sum_m C[c,b,m] * W[m,f] as matmuls and streams the result to HBM.

Precision: correctness gate is rel_err < 2e-2. Coefficients are split
hi+lo in bf16 (2-term, ~16-bit effective mantissa) and W is bf16; the
product accumulates in fp32 PSUM; output stores as bf16 (measured rel_l2
~2.3e-3 on the reference distribution). The 2-term split keeps the
contraction at K=26 <= 32, which is what unlocks the PE tiling below.

PE tiling: with K=26 the 128x128 array runs in 32x128 row-tiled mode: 4
independent matmuls (one per component) stream concurrently through the 4
row groups, quadrupling output rate vs a single K=39 matmul chain. The PE
clock is pinned at 1.2 GHz on this part (HAM never unthrottles; verified
over a 55us gapless stream), so this 4x in moving-column efficiency is the
only way to shrink PE time: stream drops ~54.6us -> ~14us of PE work.

Pipeline: per 512-col chunk-set, 4 components' matmuls fill the 4 bank
quarters of one [128,2048] PSUM tile (each quarter = exactly one bank, so
the concurrent row tiles never collide). Two such PSUM tiles ping-pong (all
8 banks). A single 2048-col PSUM->SBUF copy per set (f32->bf16 cast)
alternates DVE/ACT; sets are staged component-interleaved in SBUF and
stored to HBM in that interleaved layout (host de-interleaves for free).
The kernel is then store-bound: ~17MB of bf16 output per core at the
~310-358 GB/s per-core HBM limit. Stores are issued fine-grained early
(prime the SDMA queue the moment data exists) and coarser later.

Sharding: pure data-parallel over batch: 8 cores x 128 batches, every core
sees all 8192 freqs.
"""
import numpy as np
import sys

for _p in ("/opt/trn_rl_repo", "/root/.axon_site/_ro/trn_rl_repo"):
    if _p not in sys.path:
        sys.path.append(_p)

import ml_dtypes
import concourse.bacc as bacc
import concourse.mybir as mybir
from concourse import tile
from concourse.bass_utils import run_bass_kernel_spmd

# Problem constants (hardcoded per contract)
B, F = 1024, 8192
OP_CODES = [3, 0, 1, 2, 3, 0, 1, 2, 3, 0, 1, 2]
Q_L, Q_C = 50.0, 100.0
NK, K0 = 13, 6               # omega powers -6..+6
NCORES = 8
BPC = B // NCORES            # 128 batches per core
NCOMP = 8                    # Ar, Ai, Br, Bi, Cr, Ci, Dr, Di
OM0 = 2.0 * np.pi * np.sqrt(1e9 * 10e9)   # omega normalizer (geometric mid)

KS = 2 * NK                  # hi+lo stacked contraction dim (26)
MM_N = 512                   # moving cols per matmul (1 PSUM bank, fp32)
NSET = F // MM_N             # 16 chunk-sets per pass
NPASS = 2                    # components 0-3, then 4-7
SET_COLS = 4 * MM_N          # 2048 staged cols per set (4 comps x 512)
BF16 = ml_dtypes.bfloat16

LAST_RESULTS = None          # BassKernelResults of the most recent run
_COMPILED = {}


def _host_coeffs(values):
    """values (B,12) fp32 -> (NCOMP, B, NK) fp64 coeffs in powers of (om/OM0)."""
    v = values.astype(np.float64)
    nb = v.shape[0]
    A = np.zeros((nb, NK), np.complex128); A[:, K0] = 1.0
    Bm = np.zeros((nb, NK), np.complex128)
    Cm = np.zeros((nb, NK), np.complex128)
    Dm = np.zeros((nb, NK), np.complex128); Dm[:, K0] = 1.0

    def shift_mul(P, fac, dk):
        out = np.zeros_like(P)
        if dk == 1:
            out[:, 1:] = P[:, :-1]
        else:
            out[:, :-1] = P[:, 1:]
        return out * fac[:, None]

    for i, code in enumerate(OP_CODES):
        vi = v[:, i]
        if code == 0:      # series L
            fac = vi * OM0 * (1.0 / Q_L + 1j)
            Bm = Bm + shift_mul(A, fac, +1)
            Dm = Dm + shift_mul(Cm, fac, +1)
        elif code == 1:    # series C (reciprocal)
            c = (1.0 / Q_C - 1j) / (1.0 + 1.0 / Q_C**2)
            fac = c / (vi * OM0)
            Bm = Bm + shift_mul(A, fac, -1)
            Dm = Dm + shift_mul(Cm, fac, -1)
        elif code == 2:    # shunt L (reciprocal)
            c = (1.0 / Q_L - 1j) / (1.0 + 1.0 / Q_L**2)
            fac = c / (vi * OM0)
            A = A + shift_mul(Bm, fac, -1)
            Cm = Cm + shift_mul(Dm, fac, -1)
        else:              # shunt C
            fac = vi * OM0 * (1.0 / Q_C + 1j)
            A = A + shift_mul(Bm, fac, +1)
            Cm = Cm + shift_mul(Dm, fac, +1)
    return np.stack([A.real, A.imag, Bm.real, Bm.imag,
                     Cm.real, Cm.imag, Dm.real, Dm.imag])


# Store split schedule, in chunk-set index ranges per pass. Fine-grained at
# the start of pass 0 (prime the store pipe early) and at the very end
# (short final receipt), 2-set (1MB) pieces in the middle.
STORE_SPLITS = {
    0: [(0, 1), (1, 2), (2, 3), (3, 4), (4, 6), (6, 8),
        (8, 10), (10, 12), (12, 14), (14, 16)],
    1: [(0, 2), (2, 4), (4, 6), (6, 8), (8, 10), (10, 12),
        (12, 14), (14, 15), (15, 16)],
}


def _build_module():
    """SPMD module.

    Inputs:  cd [128, NPASS*BPC] bf16 — coefficients; partitions 32r+k hold
             hi (k<13) / lo (13<=k<26) of component 4p+r, zeros elsewhere.
             wd [128, F] bf16 — omega powers W1[k%13] replicated into each
             32-partition row group, zeros at k>=26.
    Output:  out [128, NPASS*NSET*SET_COLS] bf16, component-interleaved:
             col ((p*NSET+s)*4 + r)*512 + j  =  comp 4p+r, freq s*512+j.
    """
    nc = bacc.Bacc("TRN2", target_bir_lowering=False, debug=False,
                   enable_asserts=False, num_devices=NCORES)
    cd = nc.dram_tensor("cd", [128, NPASS * BPC], mybir.dt.bfloat16,
                        kind="ExternalInput")
    wd = nc.dram_tensor("wd", [128, F], mybir.dt.bfloat16,
                        kind="ExternalInput")
    out_d = nc.dram_tensor("out", [128, NPASS * F * 4], mybir.dt.bfloat16,
                           kind="ExternalOutput")

    with tile.TileContext(nc) as tc:
        with (
            tc.tile_pool(name="const", bufs=1) as cpool,
            tc.tile_pool(name="ps", bufs=2, space="PSUM") as pspool,
        ):
            ct = cpool.tile([128, NPASS * BPC], mybir.dt.bfloat16)
            # W chunk tiles sized so the first matmul's operands are tiny
            # and each tile's consumers only wait on their own DMA.
            w_chunks = [(0, 512), (512, 2048), (2048, 4096),
                        (4096, 6144), (6144, 8192)]
            wt = [cpool.tile([128, hi - lo], mybir.dt.bfloat16,
                             name=f"wt{ti}")
                  for ti, (lo, hi) in enumerate(w_chunks)]
            ot = cpool.tile([128, NPASS * F * 4], mybir.dt.bfloat16)

            # Head loads: only the first matmul's operands (coeffs + first
            # two W chunks, ~576KB) are issued up front, so their
            # completion semaphores land ASAP — the SDMA engines
            # round-robin every queued DMA at packet granularity, so bulk
            # W traffic queued here would delay the first chunk's last
            # per-engine semaphore increments by several us. The remaining
            # W chunks are issued from inside the loop (below), entering
            # the rings just-in-time.
            nc.sync.dma_start(ct[:, :], cd[:, :])
            nc.sync.dma_start(wt[0][:, :], wd[:, 0:512])
            nc.sync.dma_start(wt[1][:, :], wd[:, 512:2048])
            # deferred W loads: issued on the scalar ring after the ACT
            # copy of the set below, i.e. ~(set+1)*1.1us after stream start
            deferred_w = {0: 2, 2: 3, 5: 4}   # set index -> wt index

            def w_slice(s):
                col = s * MM_N
                for ti, (lo, hi) in enumerate(w_chunks):
                    if lo <= col < hi:
                        return wt[ti], col - lo
                raise AssertionError(col)

            nset_done = 0
            for p in range(NPASS):
                pend = [(a, b) for a, b in STORE_SPLITS[p]]
                for s in range(NSET):
                    # Engine-private PSUM tiles: DVE always drains ptd
                    # (comps 4p+0/1), ACT always drains pta (comps 4p+2/3).
                    # This decouples the two copy pipelines — each engine's
                    # next copy only waits on matmuls into its own tile,
                    # which ran during its previous copy, so copies stream
                    # back-to-back per engine.
                    ptd = pspool.tile([128, SET_COLS // 2], mybir.dt.float32)
                    pta = pspool.tile([128, SET_COLS // 2], mybir.dt.float32,
                                      name="pta")
                    wtile, off = w_slice(s)
                    for r in range(4):
                        # comp 4p+r on row group r -> one PSUM bank
                        pt = ptd if r < 2 else pta
                        nc.tensor.matmul(
                            pt[:, (r % 2) * MM_N:(r % 2 + 1) * MM_N],
                            ct[32 * r:32 * r + KS,
                               p * BPC:(p + 1) * BPC],
                            wtile[32 * r:32 * r + KS, off:off + MM_N],
                            tile_position=(32 * r, 0),
                        )
                    dst_lo = (p * NSET + s) * SET_COLS
                    half = SET_COLS // 2
                    nc.vector.tensor_copy(ot[:, dst_lo:dst_lo + half], ptd)
                    nc.scalar.copy(ot[:, dst_lo + half:dst_lo + SET_COLS],
                                   pta)
                    if p == 0 and s in deferred_w:
                        ti = deferred_w[s]
                        lo, hi = w_chunks[ti]
                        nc.scalar.dma_start(wt[ti][:, :], wd[:, lo:hi])
                    nset_done += 1
                    # stores stream out as soon as their sets are staged
                    if pend and s == pend[0][1] - 1:
                        a, b = pend.pop(0)
                        lo = (p * NSET + a) * SET_COLS
                        hi = (p * NSET + b) * SET_COLS
                        if p == NPASS - 1 and b == NSET:
                            # final piece: two 256KB stores on both rings so
                            # the completion receipts overlap
                            mid = (lo + hi) // 2
                            nc.sync.dma_start(out_d[:, lo:mid], ot[:, lo:mid])
                            nc.scalar.dma_start(out_d[:, mid:hi],
                                                ot[:, mid:hi])
                        else:
                            nc.sync.dma_start(out_d[:, lo:hi], ot[:, lo:hi])
    nc.compile()
    return nc


def kernel(values: np.ndarray, freq_hz: np.ndarray) -> np.ndarray:
    global LAST_RESULTS
    values = np.asarray(values, np.float32)
    freq_hz = np.asarray(freq_hz, np.float32)
    assert values.shape == (B, len(OP_CODES)) and freq_hz.shape == (F,)

    # Host precompute (tiny, fp64-exact): Laurent coefficients + omega powers
    coef = _host_coeffs(values)                              # (8, B, 13) f64
    om = 2.0 * np.pi * freq_hz.astype(np.float64)
    wt = om / OM0
    W = np.stack([wt ** (k - K0) for k in range(NK)])        # (13, F) f64
    W1 = W.astype(np.float32).astype(BF16)
    wd = np.zeros((128, F), BF16)
    for r in range(4):
        wd[32 * r:32 * r + NK] = W1
        wd[32 * r + NK:32 * r + KS] = W1
    wd = np.ascontiguousarray(wd)

    if "nc" not in _COMPILED:
        _COMPILED["nc"] = _build_module()
    nc = _COMPILED["nc"]

    in_maps = []
    for core in range(NCORES):
        sl = slice(core * BPC, (core + 1) * BPC)
        lhs = np.ascontiguousarray(
            np.transpose(coef[:, sl, :], (0, 2, 1))          # (8, 13, BPC)
        ).astype(np.float32)
        h = lhs.astype(BF16)
        lo = (lhs - h.astype(np.float32)).astype(BF16)
        cd = np.zeros((128, NPASS * BPC), BF16)
        for p in range(NPASS):
            for r in range(4):
                c = 4 * p + r
                cd[32 * r:32 * r + NK, p * BPC:(p + 1) * BPC] = h[c]
                cd[32 * r + NK:32 * r + KS, p * BPC:(p + 1) * BPC] = lo[c]
        in_maps.append({"cd": cd, "wd": wd})

    res = run_bass_kernel_spmd(nc, in_maps, core_ids=list(range(NCORES)))
    LAST_RESULTS = res
    parts = []
    for core in range(NCORES):
        dev = np.asarray(res.results[core]["out"])           # (128, 65536) bf16
        arr = dev.reshape(BPC, NPASS, NSET, 4, MM_N)
        # [b, p, s, r, j] -> [p, r, b, s, j] -> (8, BPC, F)
        parts.append(arr.transpose(1, 3, 0, 2, 4)
                     .reshape(NCOMP, BPC, F).astype(np.float32))
    return np.concatenate(parts, axis=1)


# revision 11
# speedup vs baseline: 1.1179x; 1.1179x over previous
"""Trainium2 kernel for nn_CascadedABCDCircuit: cascaded 2-port ABCD ladder.

Math: each stage multiplies the ABCD state by (I + s_i*G_i) with G_i nilpotent,
so every output component is a Laurent polynomial in omega (degree -6..+6, 13
coefficients) with batch-dependent coefficients. Host computes the coefficients
exactly in fp64 (tiny (1024,13) complex recurrence); the device evaluates
out[c,b,f] = sum_m C[c,b,m] * W[m,f] as matmuls and streams the result to HBM.

Precision: correctness gate is rel_err < 2e-2. Coefficients are split
hi+lo in bf16 (2-term, ~16-bit effective mantissa) and W is bf16; the
product accumulates in fp32 PSUM; output stores as bf16 (measured rel_l2
~2.3e-3 on the reference distribution). The 2-term split keeps the
contraction at K=26 <= 32, which is what unlocks the PE tiling below.

PE tiling: with K=26 the 128x128 array runs in 32x128 row-tiled mode: 4
independent matmuls (one per component) stream concurrently through the 4
row groups, quadrupling output rate vs a single K=39 matmul chain. The PE
clock is pinned at 1.2 GHz on this part (HAM never unthrottles; verified
over a 55us gapless stream), so this 4x in moving-column efficiency is the
only way to shrink PE time: stream drops ~54.6us -> ~14us of PE work.

Pipeline: per 512-col chunk-set, 4 components' matmuls fill the 4 bank
quarters of one [128,2048] PSUM tile (each quarter = exactly one bank, so
the concurrent row tiles never collide). Two such PSUM tiles ping-pong (all
8 banks). A single 2048-col PSUM->SBUF copy per set (f32->bf16 cast)
alternates DVE/ACT; sets are staged component-interleaved in SBUF and
stored to HBM in that interleaved layout (host de-interleaves for free).
The kernel is then store-bound: ~17MB of bf16 output per core at the
~310-358 GB/s per-core HBM limit. Stores are issued fine-grained early
(prime the SDMA queue the moment data exists) and coarser later.

Sharding: pure data-parallel over batch: 8 cores x 128 batches, every core
sees all 8192 freqs.
"""
import numpy as np
import sys

for _p in ("/opt/trn_rl_repo", "/root/.axon_site/_ro/trn_rl_repo"):
    if _p not in sys.path:
        sys.path.append(_p)

import ml_dtypes
import concourse.bacc as bacc
import concourse.mybir as mybir
from concourse import tile
from concourse.bass_utils import run_bass_kernel_spmd

# Problem constants (hardcoded per contract)
B, F = 1024, 8192
OP_CODES = [3, 0, 1, 2, 3, 0, 1, 2, 3, 0, 1, 2]
Q_L, Q_C = 50.0, 100.0
NK, K0 = 13, 6               # omega powers -6..+6
NCORES = 8
BPC = B // NCORES            # 128 batches per core
NCOMP = 8                    # Ar, Ai, Br, Bi, Cr, Ci, Dr, Di
OM0 = 2.0 * np.pi * np.sqrt(1e9 * 10e9)   # omega normalizer (geometric mid)

KS = 2 * NK                  # hi+lo stacked contraction dim (26)
MM_N = 512                   # moving cols per matmul (1 PSUM bank, fp32)
NSET = F // MM_N             # 16 chunk-sets per pass
NPASS = 2                    # components 0-3, then 4-7
SET_COLS = 4 * MM_N          # 2048 staged cols per set (4 comps x 512)
BF16 = ml_dtypes.bfloat16

LAST_RESULTS = None          # BassKernelResults of the most recent run
_COMPILED = {}


def _host_coeffs(values):
    """values (B,12) fp32 -> (NCOMP, B, NK) fp64 coeffs in powers of (om/OM0)."""
    v = values.astype(np.float64)
    nb = v.shape[0]
    A = np.zeros((nb, NK), np.complex128); A[:, K0] = 1.0
    Bm = np.zeros((nb, NK), np.complex128)
    Cm = np.zeros((nb, NK), np.complex128)
    Dm = np.zeros((nb, NK), np.complex128); Dm[:, K0] = 1.0

    def shift_mul(P, fac, dk):
        out = np.zeros_like(P)
        if dk == 1:
            out[:, 1:] = P[:, :-1]
        else:
            out[:, :-1] = P[:, 1:]
        return out * fac[:, None]

    for i, code in enumerate(OP_CODES):
        vi = v[:, i]
        if code == 0:      # series L
            fac = vi * OM0 * (1.0 / Q_L + 1j)
            Bm = Bm + shift_mul(A, fac, +1)
            Dm = Dm + shift_mul(Cm, fac, +1)
        elif code == 1:    # series C (reciprocal)
            c = (1.0 / Q_C - 1j) / (1.0 + 1.0 / Q_C**2)
            fac = c / (vi * OM0)
            Bm = Bm + shift_mul(A, fac, -1)
            Dm = Dm + shift_mul(Cm, fac, -1)
        elif code == 2:    # shunt L (reciprocal)
            c = (1.0 / Q_L - 1j) / (1.0 + 1.0 / Q_L**2)
            fac = c / (vi * OM0)
            A = A + shift_mul(Bm, fac, -1)
            Cm = Cm + shift_mul(Dm, fac, -1)
        else:              # shunt C
            fac = vi * OM0 * (1.0 / Q_C + 1j)
            A = A + shift_mul(Bm, fac, +1)
            Cm = Cm + shift_mul(Dm, fac, +1)
    return np.stack([A.real, A.imag, Bm.real, Bm.imag,
                     Cm.real, Cm.imag, Dm.real, Dm.imag])


# Store split schedule, in chunk-set index ranges per pass. Fine-grained at
# the start of pass 0 (prime the store pipe early) and at the very end
# (short final receipt), 2-set (1MB) pieces in the middle.
STORE_SPLITS = {
    0: [(0, 1), (1, 2), (2, 3), (3, 4), (4, 6), (6, 8),
        (8, 10), (10, 12), (12, 14), (14, 16)],
    1: [(0, 2), (2, 4), (4, 6), (6, 8), (8, 10), (10, 12),
        (12, 14), (14, 15), (15, 16)],
}


def _build_module():
    """SPMD module.

    Inputs:  cd [128, NPASS*BPC] bf16 — coefficients; partitions 32r+k hold
             hi (k<13) / lo (13<=k<26) of component 4p+r, zeros elsewhere.
             wd [128, F] bf16 — omega powers W1[k%13] replicated into each
             32-partition row group, zeros at k>=26.
    Output:  out [128, NPASS*NSET*SET_COLS] bf16, component-interleaved:
             col ((p*NSET+s)*4 + r)*512 + j  =  comp 4p+r, freq s*512+j.
    """
    nc = bacc.Bacc("TRN2", target_bir_lowering=False, debug=False,
                   enable_asserts=False, num_devices=NCORES)
    cd = nc.dram_tensor("cd", [128, NPASS * BPC], mybir.dt.bfloat16,
                        kind="ExternalInput")
    wd = nc.dram_tensor("wd", [128, F], mybir.dt.bfloat16,
                        kind="ExternalInput")
    out_d = nc.dram_tensor("out", [128, NPASS * F * 4], mybir.dt.bfloat16,
                           kind="ExternalOutput")

    with tile.TileContext(nc) as tc:
        with (
            tc.tile_pool(name="const", bufs=1) as cpool,
            tc.tile_pool(name="ps", bufs=2, space="PSUM") as pspool,
        ):
            ct = cpool.tile([128, NPASS * BPC], mybir.dt.bfloat16)
            # W chunk tiles sized so the first matmul's operands are tiny
            # and each tile's consumers only wait on their own DMA.
            w_chunks = [(0, 1024), (1024, 2048), (2048, 4096),
                        (4096, 6144), (6144, 8192)]
            wt = [cpool.tile([128, hi - lo], mybir.dt.bfloat16,
                             name=f"wt{ti}")
                  for ti, (lo, hi) in enumerate(w_chunks)]
            ot = cpool.tile([128, NPASS * F * 4], mybir.dt.bfloat16)

            # Input loads. The SDMA engines round-robin between the two
            # HWDGE rings at packet granularity, and a DMA's completion
            # semaphore only fires once its last per-engine packet drains,
            # so anything sharing a ring with the first chunk delays the
            # first matmul. Hence: the sync ring carries ONLY the tiny
            # critical head (coeffs + first W chunk, 320KB -> sems land
            # ~3us after issue), while all bulk W goes on the scalar ring
            # in consumption order (FIFO within the ring matches need).
            nc.sync.dma_start(ct[:, :], cd[:, :])
            nc.sync.dma_start(wt[0][:, :], wd[:, 0:1024])
            nc.scalar.dma_start(wt[1][:, :], wd[:, 1024:2048])
            nc.scalar.dma_start(wt[2][:, :], wd[:, 2048:4096])
            nc.scalar.dma_start(wt[3][:, :], wd[:, 4096:6144])
            nc.scalar.dma_start(wt[4][:, :], wd[:, 6144:8192])

            def w_slice(s):
                col = s * MM_N
                for ti, (lo, hi) in enumerate(w_chunks):
                    if lo <= col < hi:
                        return wt[ti], col - lo
                raise AssertionError(col)

            nset_done = 0
            for p in range(NPASS):
                pend = [(a, b) for a, b in STORE_SPLITS[p]]
                for s in range(NSET):
                    # Engine-private PSUM tiles: DVE always drains ptd
                    # (comps 4p+0/1), ACT always drains pta (comps 4p+2/3).
                    # This decouples the two copy pipelines — each engine's
                    # next copy only waits on matmuls into its own tile,
                    # which ran during its previous copy, so copies stream
                    # back-to-back per engine.
                    ptd = pspool.tile([128, SET_COLS // 2], mybir.dt.float32)
                    pta = pspool.tile([128, SET_COLS // 2], mybir.dt.float32,
                                      name="pta")
                    wtile, off = w_slice(s)
                    for r in range(4):
                        # comp 4p+r on row group r -> one PSUM bank
                        pt = ptd if r < 2 else pta
                        nc.tensor.matmul(
                            pt[:, (r % 2) * MM_N:(r % 2 + 1) * MM_N],
                            ct[32 * r:32 * r + KS,
                               p * BPC:(p + 1) * BPC],
                            wtile[32 * r:32 * r + KS, off:off + MM_N],
                            tile_position=(32 * r, 0),
                        )
                    dst_lo = (p * NSET + s) * SET_COLS
                    half = SET_COLS // 2
                    nc.vector.tensor_copy(ot[:, dst_lo:dst_lo + half], ptd)
                    nc.scalar.copy(ot[:, dst_lo + half:dst_lo + SET_COLS],
                                   pta)
                    nset_done += 1
                    # stores stream out as soon as their sets are staged
                    if pend and s == pend[0][1] - 1:
                        a, b = pend.pop(0)
                        lo = (p * NSET + a) * SET_COLS
                        hi = (p * NSET + b) * SET_COLS
                        if p == NPASS - 1 and b == NSET:
                            # final piece: two 256KB stores on both rings so
                            # the completion receipts overlap
                            mid = (lo + hi) // 2
                            nc.sync.dma_start(out_d[:, lo:mid], ot[:, lo:mid])
                            nc.scalar.dma_start(out_d[:, mid:hi],
                                                ot[:, mid:hi])
                        else:
                            nc.sync.dma_start(out_d[:, lo:hi], ot[:, lo:hi])
    nc.compile()
    return nc


def kernel(values: np.ndarray, freq_hz: np.ndarray) -> np.ndarray:
    global LAST_RESULTS
    values = np.asarray(values, np.float32)
    freq_hz = np.asarray(freq_hz, np.float32)
    assert values.shape == (B, len(OP_CODES)) and freq_hz.shape == (F,)

    # Host precompute (tiny, fp64-exact): Laurent coefficients + omega powers
    coef = _host_coeffs(values)                              # (8, B, 13) f64
    om = 2.0 * np.pi * freq_hz.astype(np.float64)
    wt = om / OM0
    W = np.stack([wt ** (k - K0) for k in range(NK)])        # (13, F) f64
    W1 = W.astype(np.float32).astype(BF16)
    wd = np.zeros((128, F), BF16)
    for r in range(4):
        wd[32 * r:32 * r + NK] = W1
        wd[32 * r + NK:32 * r + KS] = W1
    wd = np.ascontiguousarray(wd)

    if "nc" not in _COMPILED:
        _COMPILED["nc"] = _build_module()
    nc = _COMPILED["nc"]

    in_maps = []
    for core in range(NCORES):
        sl = slice(core * BPC, (core + 1) * BPC)
        lhs = np.ascontiguousarray(
            np.transpose(coef[:, sl, :], (0, 2, 1))          # (8, 13, BPC)
        ).astype(np.float32)
        h = lhs.astype(BF16)
        lo = (lhs - h.astype(np.float32)).astype(BF16)
        cd = np.zeros((128, NPASS * BPC), BF16)
        for p in range(NPASS):
            for r in range(4):
                c = 4 * p + r
                cd[32 * r:32 * r + NK, p * BPC:(p + 1) * BPC] = h[c]
                cd[32 * r + NK:32 * r + KS, p * BPC:(p + 1) * BPC] = lo[c]
        in_maps.append({"cd": cd, "wd": wd})

    res = run_bass_kernel_spmd(nc, in_maps, core_ids=list(range(NCORES)))
    LAST_RESULTS = res
    parts = []
    for core in range(NCORES):
        dev = np.asarray(res.results[core]["out"])           # (128, 65536) bf16
        arr = dev.reshape(BPC, NPASS, NSET, 4, MM_N)
        # [b, p, s, r, j] -> [p, r, b, s, j] -> (8, BPC, F)
        parts.append(arr.transpose(1, 3, 0, 2, 4)
                     .reshape(NCOMP, BPC, F).astype(np.float32))
    return np.concatenate(parts, axis=1)


# revision 17
# speedup vs baseline: 1.1586x; 1.0364x over previous
"""Trainium2 kernel for nn_CascadedABCDCircuit: cascaded 2-port ABCD ladder.

Math: each stage multiplies the ABCD state by (I + s_i*G_i) with G_i nilpotent,
so every output component is a Laurent polynomial in omega (degree -6..+6, 13
coefficients) with batch-dependent coefficients. Host computes the coefficients
exactly in fp64 (tiny (1024,13) complex recurrence); the device evaluates
out[c,b,f] = sum_m C[c,b,m] * W[m,f] as matmuls and streams the result to HBM.

Precision: correctness gate is rel_err < 2e-2. Coefficients are split
hi+lo in bf16 (2-term, ~16-bit effective mantissa) and W is bf16; the
product accumulates in fp32 PSUM; output stores as bf16 (measured rel_l2
~2.3e-3 on the reference distribution). The 2-term split keeps the
contraction at K=26 <= 32, which is what unlocks the PE tiling below.

PE tiling: with K=26 the 128x128 array runs in 32x128 row-tiled mode: 4
independent matmuls (one per component) stream concurrently through the 4
row groups, quadrupling output rate vs a single K=39 matmul chain. The PE
clock is pinned at 1.2 GHz on this part (HAM never unthrottles; verified
over a 55us gapless stream), so this 4x in moving-column efficiency is the
only way to shrink PE time: stream drops ~54.6us -> ~14us of PE work.

Pipeline: per 512-col chunk-set, 4 components' matmuls fill the 4 bank
quarters of one [128,2048] PSUM tile (each quarter = exactly one bank, so
the concurrent row tiles never collide). Two such PSUM tiles ping-pong (all
8 banks). A single 2048-col PSUM->SBUF copy per set (f32->bf16 cast)
alternates DVE/ACT; sets are staged component-interleaved in SBUF and
stored to HBM in that interleaved layout (host de-interleaves for free).
The kernel is then store-bound: ~17MB of bf16 output per core at the
~310-358 GB/s per-core HBM limit. Stores are issued fine-grained early
(prime the SDMA queue the moment data exists) and coarser later.

Sharding: pure data-parallel over batch: 8 cores x 128 batches, every core
sees all 8192 freqs.
"""
import numpy as np
import sys

for _p in ("/opt/trn_rl_repo", "/root/.axon_site/_ro/trn_rl_repo"):
    if _p not in sys.path:
        sys.path.append(_p)

import ml_dtypes
import concourse.bacc as bacc
import concourse.mybir as mybir
from concourse import tile
from concourse.bass_utils import run_bass_kernel_spmd

# Problem constants (hardcoded per contract)
B, F = 1024, 8192
OP_CODES = [3, 0, 1, 2, 3, 0, 1, 2, 3, 0, 1, 2]
Q_L, Q_C = 50.0, 100.0
NK, K0 = 13, 6               # omega powers -6..+6
NCORES = 8
BPC = B // NCORES            # 128 batches per core
NCOMP = 8                    # Ar, Ai, Br, Bi, Cr, Ci, Dr, Di
OM0 = 2.0 * np.pi * np.sqrt(1e9 * 10e9)   # omega normalizer (geometric mid)

KS = 2 * NK                  # hi+lo stacked contraction dim (26)
MM_N = 512                   # moving cols per matmul (1 PSUM bank, fp32)
NSET = F // MM_N             # 16 chunk-sets per pass
NPASS = 2                    # components 0-3, then 4-7
SET_COLS = 4 * MM_N          # 2048 staged cols per set (4 comps x 512)
BF16 = ml_dtypes.bfloat16

LAST_RESULTS = None          # BassKernelResults of the most recent run
_COMPILED = {}


def _host_coeffs(values):
    """values (B,12) fp32 -> (NCOMP, B, NK) fp64 coeffs in powers of (om/OM0)."""
    v = values.astype(np.float64)
    nb = v.shape[0]
    A = np.zeros((nb, NK), np.complex128); A[:, K0] = 1.0
    Bm = np.zeros((nb, NK), np.complex128)
    Cm = np.zeros((nb, NK), np.complex128)
    Dm = np.zeros((nb, NK), np.complex128); Dm[:, K0] = 1.0

    def shift_mul(P, fac, dk):
        out = np.zeros_like(P)
        if dk == 1:
            out[:, 1:] = P[:, :-1]
        else:
            out[:, :-1] = P[:, 1:]
        return out * fac[:, None]

    for i, code in enumerate(OP_CODES):
        vi = v[:, i]
        if code == 0:      # series L
            fac = vi * OM0 * (1.0 / Q_L + 1j)
            Bm = Bm + shift_mul(A, fac, +1)
            Dm = Dm + shift_mul(Cm, fac, +1)
        elif code == 1:    # series C (reciprocal)
            c = (1.0 / Q_C - 1j) / (1.0 + 1.0 / Q_C**2)
            fac = c / (vi * OM0)
            Bm = Bm + shift_mul(A, fac, -1)
            Dm = Dm + shift_mul(Cm, fac, -1)
        elif code == 2:    # shunt L (reciprocal)
            c = (1.0 / Q_L - 1j) / (1.0 + 1.0 / Q_L**2)
            fac = c / (vi * OM0)
            A = A + shift_mul(Bm, fac, -1)
            Cm = Cm + shift_mul(Dm, fac, -1)
        else:              # shunt C
            fac = vi * OM0 * (1.0 / Q_C + 1j)
            A = A + shift_mul(Bm, fac, +1)
            Cm = Cm + shift_mul(Dm, fac, +1)
    return np.stack([A.real, A.imag, Bm.real, Bm.imag,
                     Cm.real, Cm.imag, Dm.real, Dm.imag])


# Store split schedule, in chunk-set index ranges per pass: one 512KB store
# per set, issued the moment its copies land — keeps the SDMA queue fed at
# the production rate with no ramp-up starvation.
STORE_SPLITS = {
    0: [(s, s + 1) for s in range(NSET)],
    1: [(s, s + 1) for s in range(NSET)],
}


def _build_module():
    """SPMD module.

    Inputs:  cd [128, NPASS*BPC] bf16 — coefficients; partitions 32r+k hold
             hi (k<13) / lo (13<=k<26) of component 4p+r, zeros elsewhere.
             wd [128, F] bf16 — omega powers W1[k%13] replicated into each
             32-partition row group, zeros at k>=26.
    Output:  out [128, NPASS*NSET*SET_COLS] bf16, component-interleaved:
             col ((p*NSET+s)*4 + r)*512 + j  =  comp 4p+r, freq s*512+j.
    """
    nc = bacc.Bacc("TRN2", target_bir_lowering=False, debug=False,
                   enable_asserts=False, num_devices=NCORES)
    # head tensor: coeffs (256 cols) + first W chunk (1024 cols) packed in
    # one dram tensor so the critical first load is a single DMA with a
    # single completion semaphore
    hd = nc.dram_tensor("hd", [128, NPASS * BPC + 1024], mybir.dt.bfloat16,
                        kind="ExternalInput")
    wd = nc.dram_tensor("wd", [128, F - 1024], mybir.dt.bfloat16,
                        kind="ExternalInput")
    out_d = nc.dram_tensor("out", [128, NPASS * F * 4], mybir.dt.bfloat16,
                           kind="ExternalOutput")

    with tile.TileContext(nc) as tc:
        with (
            tc.tile_pool(name="const", bufs=1) as cpool,
            tc.tile_pool(name="ps", bufs=2, space="PSUM") as pspool,
        ):
            # head tile: coeffs + first W chunk in one allocation
            ht = cpool.tile([128, NPASS * BPC + 1024], mybir.dt.bfloat16)
            ct = ht[:, :NPASS * BPC]
            # W chunk tiles sized so the first matmul's operands are tiny
            # and each tile's consumers only wait on their own DMA.
            w_chunks = [(1024, 2048), (2048, 4096),
                        (4096, 6144), (6144, 8192)]
            wt = [cpool.tile([128, hi - lo], mybir.dt.bfloat16,
                             name=f"wt{ti}")
                  for ti, (lo, hi) in enumerate(w_chunks)]
            ot = cpool.tile([128, NPASS * F * 4], mybir.dt.bfloat16)

            # Input loads. The SDMA engines round-robin between the two
            # HWDGE rings at packet granularity, and a DMA's completion
            # semaphore only fires once its last per-engine packet drains,
            # so anything sharing a ring with the first chunk delays the
            # first matmul. Hence: the sync ring carries ONLY the critical
            # head (coeffs + first W chunk, one 320KB DMA -> one semaphore
            # landing ASAP), while all bulk W goes on the scalar ring in
            # consumption order (FIFO within the ring matches need).
            nc.sync.dma_start(ht[:, :], hd[:, :])
            nc.scalar.dma_start(wt[0][:, :], wd[:, 0:1024])
            nc.scalar.dma_start(wt[1][:, :], wd[:, 1024:3072])
            nc.scalar.dma_start(wt[2][:, :], wd[:, 3072:5120])
            nc.scalar.dma_start(wt[3][:, :], wd[:, 5120:7168])

            def w_slice(s):
                col = s * MM_N
                if col < 1024:
                    return ht, NPASS * BPC + col
                for ti, (lo, hi) in enumerate(w_chunks):
                    if lo <= col < hi:
                        return wt[ti], col - lo
                raise AssertionError(col)

            nset_done = 0
            for p in range(NPASS):
                pend = [(a, b) for a, b in STORE_SPLITS[p]]
                for s in range(NSET):
                    # Engine-private PSUM tiles: DVE always drains ptd
                    # (comps 4p+0/1), ACT always drains pta (comps 4p+2/3).
                    # This decouples the two copy pipelines — each engine's
                    # next copy only waits on matmuls into its own tile,
                    # which ran during its previous copy, so copies stream
                    # back-to-back per engine.
                    ptd = pspool.tile([128, SET_COLS // 2], mybir.dt.float32)
                    pta = pspool.tile([128, SET_COLS // 2], mybir.dt.float32,
                                      name="pta")
                    wtile, off = w_slice(s)
                    for r in range(4):
                        # comp 4p+r on row group r -> one PSUM bank
                        pt = ptd if r < 2 else pta
                        nc.tensor.matmul(
                            pt[:, (r % 2) * MM_N:(r % 2 + 1) * MM_N],
                            ct[32 * r:32 * r + KS,
                               p * BPC:(p + 1) * BPC],
                            wtile[32 * r:32 * r + KS, off:off + MM_N],
                            tile_position=(32 * r, 0),
                        )
                    dst_lo = (p * NSET + s) * SET_COLS
                    half = SET_COLS // 2
                    nc.vector.tensor_copy(ot[:, dst_lo:dst_lo + half], ptd)
                    nc.scalar.copy(ot[:, dst_lo + half:dst_lo + SET_COLS],
                                   pta)
                    nset_done += 1
                    # stores stream out as soon as their sets are staged
                    if pend and s == pend[0][1] - 1:
                        a, b = pend.pop(0)
                        lo = (p * NSET + a) * SET_COLS
                        hi = (p * NSET + b) * SET_COLS
                        if p == NPASS - 1 and b == NSET:
                            # final piece: two 256KB stores on both rings so
                            # the completion receipts overlap
                            mid = (lo + hi) // 2
                            nc.sync.dma_start(out_d[:, lo:mid], ot[:, lo:mid])
                            nc.scalar.dma_start(out_d[:, mid:hi],
                                                ot[:, mid:hi])
                        else:
                            nc.sync.dma_start(out_d[:, lo:hi], ot[:, lo:hi])
    nc.compile()
    return nc


def kernel(values: np.ndarray, freq_hz: np.ndarray) -> np.ndarray:
    global LAST_RESULTS
    values = np.asarray(values, np.float32)
    freq_hz = np.asarray(freq_hz, np.float32)
    assert values.shape == (B, len(OP_CODES)) and freq_hz.shape == (F,)

    # Host precompute (tiny, fp64-exact): Laurent coefficients + omega powers
    coef = _host_coeffs(values)                              # (8, B, 13) f64
    om = 2.0 * np.pi * freq_hz.astype(np.float64)
    wt = om / OM0
    W = np.stack([wt ** (k - K0) for k in range(NK)])        # (13, F) f64
    W1 = W.astype(np.float32).astype(BF16)
    wfull = np.zeros((128, F), BF16)
    for r in range(4):
        wfull[32 * r:32 * r + NK] = W1
        wfull[32 * r + NK:32 * r + KS] = W1
    wd = np.ascontiguousarray(wfull[:, 1024:])

    if "nc" not in _COMPILED:
        _COMPILED["nc"] = _build_module()
    nc = _COMPILED["nc"]

    in_maps = []
    for core in range(NCORES):
        sl = slice(core * BPC, (core + 1) * BPC)
        lhs = np.ascontiguousarray(
            np.transpose(coef[:, sl, :], (0, 2, 1))          # (8, 13, BPC)
        ).astype(np.float32)
        h = lhs.astype(BF16)
        lo = (lhs - h.astype(np.float32)).astype(BF16)
        hdv = np.zeros((128, NPASS * BPC + 1024), BF16)
        for p in range(NPASS):
            for r in range(4):
                c = 4 * p + r
                hdv[32 * r:32 * r + NK, p * BPC:(p + 1) * BPC] = h[c]
                hdv[32 * r + NK:32 * r + KS, p * BPC:(p + 1) * BPC] = lo[c]
        hdv[:, NPASS * BPC:] = wfull[:, :1024]
        in_maps.append({"hd": hdv, "wd": wd})

    res = run_bass_kernel_spmd(nc, in_maps, core_ids=list(range(NCORES)))
    LAST_RESULTS = res
    parts = []
    for core in range(NCORES):
        dev = np.asarray(res.results[core]["out"])           # (128, 65536) bf16
        arr = dev.reshape(BPC, NPASS, NSET, 4, MM_N)
        # [b, p, s, r, j] -> [p, r, b, s, j] -> (8, BPC, F)
        parts.append(arr.transpose(1, 3, 0, 2, 4)
                     .reshape(NCOMP, BPC, F).astype(np.float32))
    return np.concatenate(parts, axis=1)
